# revision 30
# baseline (speedup 1.0000x reference)
"""Trainium2 Bass kernel for nn_CorrelationMapLayer.

reference semantics:
    d1 = bilinear_down28(feature1)            # [B, C, 28, 28]
    d2 = bilinear_down28(feature2)            # [B, C, 28, 28]
    f2_sel[b,c,k] = d2[b, c, y_k, x_k]        # knn gather (y=knn[:,1], x=knn[:,0])
    corr = relu(einsum('bck,bchw->bkhw', f2_sel, d1))
    out  = corr / sum_{h,w} exp(corr) * 10

Kernel structure (v8):
  * inputs are cast to bf16 on the host; DMA is the roofline.
  * f2 is consumed only through the 4 bilinear taps at the K knn points
    (0.5% of the tensor). The host-side shard step slices those tap
    columns out of f2 (pure indexing -- the bilinear weights and all
    arithmetic stay on device), so each core loads ~13MB instead of ~26MB.
  * d2sel [c, K] = weighted combine of the 4 taps (DVE: one mul + two
    pair adds per channel block).
  * f1 feeds the correlation matmul in the original 56x56 space as raw
    bf16 (no elementwise work); the bilinear downsample is applied AFTER
    the matmul on corr56 [K=100, 3136] (K < C so this is cheap): psum ->
    bf16 copy, premultiply by the separable weight map, h-pair add,
    strided w-pair add, then relu / exp+accumulate / reciprocal / scale.
  * Queues: input DMAs on the SP HWDGE queue, output on the ACT HWDGE
    queue so neither stream head-of-line blocks the other.
  * Data parallel over batch: 4 batches per core x 8 cores.
"""

import os
import sys

import numpy as np

for _p in (
    "/root/.axon_site",
    "/root/.axon_site/_ro/trn_rl_repo",
    "/root/.axon_site/_ro/pypackages",
    "/opt/trn_rl_repo",
):
    if os.path.isdir(_p) and _p not in sys.path:
        sys.path.append(_p)

import concourse.bacc as bacc
import concourse.mybir as mybir
import concourse.tile as tile
from concourse import bass_utils

F32 = mybir.dt.float32
BF16 = mybir.dt.bfloat16
FP8 = mybir.dt.float8e4
F16 = mybir.dt.float16
F16_NP = mybir.dt.np(mybir.dt.float16)
FP8_NP = mybir.dt.np(mybir.dt.float8e4)
F1SCALE = 16.0
AF = mybir.ActivationFunctionType

B, C, H, W, K = 32, 512, 56, 56, 100
NCORES = 8
BL = B // NCORES  # batches per core
S = 28
HW = H * W  # 3136
HW28 = S * S  # 784
NCB = C // 128  # 4 channel blocks
NG = 7  # corr h-row groups (7 x 8 rows)

BF16_NP = mybir.dt.np(BF16)


def _bilinear_matrix(in_size: int, out_size: int) -> np.ndarray:
    scale = np.float32((in_size - 1) / (out_size - 1)) if out_size > 1 else np.float32(0)
    coords = np.arange(out_size, dtype=np.float32) * scale
    lo = np.floor(coords).astype(np.int32)
    hi = np.minimum(lo + 1, in_size - 1)
    frac = coords - lo.astype(np.float32)
    M = np.zeros((out_size, in_size), np.float32)
    np.add.at(M, (np.arange(out_size), lo), np.float32(1.0) - frac)
    np.add.at(M, (np.arange(out_size), hi), frac)
    return M


def _tap_weights() -> np.ndarray:
    """wvec[w]: weight applied to input index w, whose (unique) consumer is
    output index w//2. Verifies the 2-tap stride-2 structure exactly."""
    M = _bilinear_matrix(H, S)  # [28, 56]
    wvec = np.zeros(H, np.float32)
    for w in range(H):
        wvec[w] = M[w // 2, w]
    M2 = np.zeros_like(M)
    for ow in range(S):
        M2[ow, 2 * ow] = wvec[2 * ow]
        M2[ow, 2 * ow + 1] = wvec[2 * ow + 1]
    assert np.abs(M - M2).max() <= 1e-6, "bilinear 2-tap structure violated"
    return wvec


_WVEC = _tap_weights()
# WF[p, h*56+w] = wvec[h]*wvec[w]  (full separable 2D weight map)
_WF_ROW = (np.repeat(_WVEC, W) * np.tile(_WVEC, H)).astype(np.float32)
# the corr path premultiply also undoes the x16 f1 prescale
WF_NP = np.ascontiguousarray(
    np.broadcast_to(_WF_ROW[None, :] / 16.0, (128, HW)), dtype=F16_NP
)


def _tap_tables(knn_inds: np.ndarray):
    """Flat hw indices of the 4 bilinear taps per knn point (for the host
    slice) + the matching tap-weight map (applied on device)."""
    knn = np.asarray(knn_inds)
    taps = np.zeros(4 * K, np.int64)
    wtap = np.zeros((128, 4 * K), np.float32)
    for k in range(knn.shape[0]):
        x = int(knn[k, 0])
        y = int(knn[k, 1])
        for j, (t, s) in enumerate(((0, 0), (0, 1), (1, 0), (1, 1))):
            taps[4 * k + j] = (2 * y + t) * W + (2 * x + s)
            wtap[:, 4 * k + j] = _WVEC[2 * y + t] * _WVEC[2 * x + s]
    return taps, np.ascontiguousarray(wtap.astype(F16_NP))


def _make_in_maps(f1: np.ndarray, f2: np.ndarray, knn_inds: np.ndarray):
    taps, wtap = _tap_tables(knn_inds)
    # mixed-precision f1: channel blocks 0-1 ship as f16, blocks 2-3 as
    # fp8e4m3 (quantization noise scales with sqrt of the fp8 fraction).
    # Both are prescaled by 16 (exact in f16) so the psum scale is uniform;
    # the 1/16 is folded into the psum->f16 copy scale on device.
    f1_32 = np.asarray(f1, np.float32) * F1SCALE
    f1hi = f1_32[:, : C // 2].astype(F16_NP)
    f1lo = f1_32[:, C // 2 :].astype(FP8_NP)
    # host-side shard/slice: tap columns of f2 (indexing only, no math);
    # sliced from the original f32 values and shipped as f16
    f2t = np.ascontiguousarray(
        np.asarray(f2, np.float32).reshape(B, C, HW)[:, :, taps].astype(F16_NP)
    )
    in_maps = []
    for c in range(NCORES):
        in_maps.append(
            {
                "f1hi": np.ascontiguousarray(f1hi[c * BL : (c + 1) * BL]),
                "f1lo": np.ascontiguousarray(f1lo[c * BL : (c + 1) * BL]),
                "f2t": f2t[c * BL : (c + 1) * BL],
                "wf": WF_NP,
                "wtap": wtap,
            }
        )
    return in_maps


def _build(tc, out_ap, f1hi_ap, f1lo_ap, f2t_ap, wf_ap, wtap_ap):
    nc = tc.nc
    MS = __import__("concourse.bass", fromlist=["MemorySpace"]).MemorySpace

    from contextlib import ExitStack

    with ExitStack() as ctx:
        const = ctx.enter_context(tc.tile_pool(name="const", bufs=1))
        f2tp = ctx.enter_context(tc.tile_pool(name="f2tp", bufs=4))
        mpool = ctx.enter_context(tc.tile_pool(name="mpool", bufs=2))
        s1p = ctx.enter_context(tc.tile_pool(name="s1p", bufs=2))
        d2selp = ctx.enter_context(tc.tile_pool(name="d2selp", bufs=16))
        tf1p = ctx.enter_context(tc.tile_pool(name="tf1p", bufs=6))
        cbp = ctx.enter_context(tc.tile_pool(name="cbp", bufs=3))
        cbhp = ctx.enter_context(tc.tile_pool(name="cbhp", bufs=3))
        up = ctx.enter_context(tc.tile_pool(name="up", bufs=3))
        vp = ctx.enter_context(tc.tile_pool(name="vp", bufs=3))
        crawp = ctx.enter_context(tc.tile_pool(name="crawp", bufs=2))
        c28p = ctx.enter_context(tc.tile_pool(name="c28p", bufs=2))
        expbp = ctx.enter_context(tc.tile_pool(name="expbp", bufs=2))
        obp = ctx.enter_context(tc.tile_pool(name="obp", bufs=2))
        smallp = ctx.enter_context(tc.tile_pool(name="smallp", bufs=6))
        cpsph = ctx.enter_context(tc.tile_pool(name="cpsph", bufs=4, space=MS.PSUM))
        cpspl = ctx.enter_context(tc.tile_pool(name="cpspl", bufs=4, space=MS.PSUM))

        wf = const.tile([128, HW], F16, tag="wf")
        wtap = const.tile([128, 4 * K], F16, tag="wtap")
        nc.scalar.dma_start(wf[:], wf_ap)
        nc.scalar.dma_start(wtap[:], wtap_ap)

        # ---- f2 taps for ALL batches up front: tiny DMAs + combines, so
        # the PE matmul stream later only ever waits on f1 quarters ----
        d2sel_all = []
        for b in range(BL):
            f2t = f2tp.tile([128, NCB * 4 * K], F16, tag="f2t")
            nc.sync.dma_start(
                f2t.rearrange("c (g x) -> c g x", g=NCB),
                f2t_ap[b].rearrange("(g c) x -> c g x", c=128),
            )
            f2tv = f2t.rearrange("c (g x) -> c g x", g=NCB)
            d2sel_tiles = []
            for i in range(NCB):
                m = mpool.tile([128, 4 * K], F16, tag="m")
                nc.vector.tensor_mul(m[:], f2tv[:, i, :], wtap[:])
                mv = m.rearrange("c (k d) -> c k d", d=2)
                s1 = s1p.tile([128, 2 * K], F16, tag="s1")
                nc.vector.tensor_add(s1[:], mv[:, :, 0], mv[:, :, 1])
                s1v = s1.rearrange("c (k d) -> c k d", d=2)
                dsel = d2selp.tile([128, K], F16, tag="d2sel")
                nc.vector.tensor_add(dsel[:], s1v[:, :, 0], s1v[:, :, 1])
                d2sel_tiles.append(dsel)
            d2sel_all.append(d2sel_tiles)

        for b in range(BL):
            d2sel_tiles = d2sel_all[b]

            # ---- f1 load: quarter-DMAs (16/16/16/8 h rows), channel
            # blocks 0-1 in f16 and 2-3 in fp8 ----
            tf1q = []
            for q, (r0, nr) in enumerate([(0, 16), (16, 16), (32, 16), (48, 8)]):
                thi = tf1p.tile([128, 2 * nr * W], F16, tag=f"tf1qh{q}")
                nc.sync.dma_start(
                    thi.rearrange("c (g x) -> c g x", g=2),
                    f1hi_ap[b, :, r0 : r0 + nr, :].rearrange(
                        "(g c) h w -> c g (h w)", c=128
                    ),
                )
                tlo = tf1p.tile([128, 2 * nr * W], FP8, tag=f"tf1ql{q}")
                nc.sync.dma_start(
                    tlo.rearrange("c (g x) -> c g x", g=2),
                    f1lo_ap[b, :, r0 : r0 + nr, :].rearrange(
                        "(g c) h w -> c g (h w)", c=128
                    ),
                )
                tf1q.append(
                    (
                        thi.rearrange("c (g h w) -> c g h w", g=2, h=nr),
                        tlo.rearrange("c (g h w) -> c g h w", g=2, h=nr),
                    )
                )

            # ---- correlation in 56x56 space + post-matmul downsample ----
            craw = crawp.tile([K, HW28], F32, tag="craw")
            cr3 = craw.rearrange("k (h w) -> k h w", h=S)
            for g in range(NG):  # 7 groups of 8 h-rows
                h0 = g * 8
                tvh, tvl = tf1q[g // 2]
                hh = (g % 2) * 8
                cpsh = cpsph.tile([K, 8 * W], F32, tag="cpsh")
                cpsl = cpspl.tile([K, 8 * W], F32, tag="cpsl")
                for i in range(2):
                    nc.tensor.matmul(
                        cpsh[:],
                        d2sel_tiles[i][:],
                        tvh[:, i, hh : hh + 8, :],
                        start=(i == 0),
                        stop=(i == 1),
                    )
                for i in range(2):
                    nc.tensor.matmul(
                        cpsl[:],
                        d2sel_tiles[2 + i][:],
                        tvl[:, i, hh : hh + 8, :],
                        start=(i == 0),
                        stop=(i == 1),
                    )
                cbh = cbhp.tile([K, 8 * W], F16, tag="cbh")
                nc.scalar.copy(cbh[:], cpsh[:])
                cb = cbp.tile([K, 8 * W], F16, tag="cb")
                nc.vector.scalar_tensor_tensor(
                    cb[:], cbh[:], 1.0, cpsl[:],
                    mybir.AluOpType.mult, mybir.AluOpType.add,
                )
                deng = nc.gpsimd if g in (2, 5) else nc.vector
                u = up.tile([K, 8 * W], F16, tag="u")
                deng.tensor_mul(u[:], cb[:], wf[0:K, h0 * W : (h0 + 8) * W])
                u3 = u.rearrange("k (h w) -> k h w", h=8)
                v = vp.tile([K, 4 * W], F16, tag="v")
                v3 = v.rearrange("k (h w) -> k h w", h=4)
                deng.tensor_add(v3, u3[:, 0:8:2, :], u3[:, 1:8:2, :])
                deng.tensor_add(
                    cr3[:, g * 4 : (g + 1) * 4, :],
                    v3[:, :, 0:W:2],
                    v3[:, :, 1:W:2],
                )

            # ---- relu, exp + accumulate, reciprocal, scale by 10/denom ----
            c28 = c28p.tile([K, HW28], F32, tag="c28")
            nc.scalar.activation(c28[:], craw[:], AF.Relu)
            expb = expbp.tile([K, HW28], BF16, tag="expb")
            den = smallp.tile([K, 1], F32, tag="den")
            nc.scalar.activation(expb[:], c28[:], AF.Exp, accum_out=den[:])
            rec = smallp.tile([K, 1], F32, tag="rec")
            nc.vector.reciprocal(rec[:], den[:])
            rec10 = smallp.tile([K, 1], F32, tag="rec10")
            nc.vector.tensor_scalar_mul(rec10[:], rec[:], 10.0)
            ob = obp.tile([K, HW28], F32, tag="ob")
            nc.scalar.mul(ob[:], c28[:], rec10[:])
            # out DMA on the ACT HWDGE queue (inputs stream on SP)
            nc.scalar.dma_start(out_ap[b], ob[:])


_CACHE: dict = {}


def _get_nc():
    if "nc" in _CACHE:
        return _CACHE["nc"]
    nc = bacc.Bacc(
        "TRN2",
        target_bir_lowering=False,
        debug=False,
        enable_asserts=False,
        num_devices=NCORES,
    )
    f1hi = nc.dram_tensor("f1hi", [BL, C // 2, H, W], F16, kind="ExternalInput").ap()
    f1lo = nc.dram_tensor("f1lo", [BL, C // 2, H, W], FP8, kind="ExternalInput").ap()
    f2t = nc.dram_tensor("f2t", [BL, C, 4 * K], F16, kind="ExternalInput").ap()
    wf = nc.dram_tensor("wf", [128, HW], F16, kind="ExternalInput").ap()
    wtap = nc.dram_tensor("wtap", [128, 4 * K], F16, kind="ExternalInput").ap()
    out = nc.dram_tensor("out", [BL, K, HW28], F32, kind="ExternalOutput").ap()
    with tile.TileContext(nc) as tc:
        _build(tc, out, f1hi, f1lo, f2t, wf, wtap)
    nc.compile()
    _CACHE["nc"] = nc
    return nc


def kernel(feature1, feature2, knn_inds):
    f1 = np.asarray(feature1, dtype=np.float32)
    f2 = np.asarray(feature2, dtype=np.float32)
    nc = _get_nc()
    in_maps = _make_in_maps(f1, f2, knn_inds)
    res = bass_utils.run_bass_kernel_spmd(nc, in_maps, core_ids=list(range(NCORES)))
    _CACHE["last_results"] = res
    out = np.concatenate([r["out"] for r in res.results], axis=0)
    return out.reshape(B, K, S, S)


# revision 32
# speedup vs baseline: 1.0011x; 1.0011x over previous
"""Trainium2 Bass kernel for nn_CorrelationMapLayer.

reference semantics:
    d1 = bilinear_down28(feature1)            # [B, C, 28, 28]
    d2 = bilinear_down28(feature2)            # [B, C, 28, 28]
    f2_sel[b,c,k] = d2[b, c, y_k, x_k]        # knn gather (y=knn[:,1], x=knn[:,0])
    corr = relu(einsum('bck,bchw->bkhw', f2_sel, d1))
    out  = corr / sum_{h,w} exp(corr) * 10

Kernel structure (v8):
  * inputs are cast to bf16 on the host; DMA is the roofline.
  * f2 is consumed only through the 4 bilinear taps at the K knn points
    (0.5% of the tensor). The host-side shard step slices those tap
    columns out of f2 (pure indexing -- the bilinear weights and all
    arithmetic stay on device), so each core loads ~13MB instead of ~26MB.
  * d2sel [c, K] = weighted combine of the 4 taps (DVE: one mul + two
    pair adds per channel block).
  * f1 feeds the correlation matmul in the original 56x56 space as raw
    bf16 (no elementwise work); the bilinear downsample is applied AFTER
    the matmul on corr56 [K=100, 3136] (K < C so this is cheap): psum ->
    bf16 copy, premultiply by the separable weight map, h-pair add,
    strided w-pair add, then relu / exp+accumulate / reciprocal / scale.
  * Queues: input DMAs on the SP HWDGE queue, output on the ACT HWDGE
    queue so neither stream head-of-line blocks the other.
  * Data parallel over batch: 4 batches per core x 8 cores.
"""

import os
import sys

import numpy as np

for _p in (
    "/root/.axon_site",
    "/root/.axon_site/_ro/trn_rl_repo",
    "/root/.axon_site/_ro/pypackages",
    "/opt/trn_rl_repo",
):
    if os.path.isdir(_p) and _p not in sys.path:
        sys.path.append(_p)

import concourse.bacc as bacc
import concourse.mybir as mybir
import concourse.tile as tile
from concourse import bass_utils

F32 = mybir.dt.float32
BF16 = mybir.dt.bfloat16
FP8 = mybir.dt.float8e4
F16 = mybir.dt.float16
F16_NP = mybir.dt.np(mybir.dt.float16)
FP8_NP = mybir.dt.np(mybir.dt.float8e4)
F1SCALE = 1.0
AF = mybir.ActivationFunctionType

B, C, H, W, K = 32, 512, 56, 56, 100
NCORES = 8
BL = B // NCORES  # batches per core
S = 28
HW = H * W  # 3136
HW28 = S * S  # 784
NCB = C // 128  # 4 channel blocks
NG = 7  # corr h-row groups (7 x 8 rows)

BF16_NP = mybir.dt.np(BF16)


def _bilinear_matrix(in_size: int, out_size: int) -> np.ndarray:
    scale = np.float32((in_size - 1) / (out_size - 1)) if out_size > 1 else np.float32(0)
    coords = np.arange(out_size, dtype=np.float32) * scale
    lo = np.floor(coords).astype(np.int32)
    hi = np.minimum(lo + 1, in_size - 1)
    frac = coords - lo.astype(np.float32)
    M = np.zeros((out_size, in_size), np.float32)
    np.add.at(M, (np.arange(out_size), lo), np.float32(1.0) - frac)
    np.add.at(M, (np.arange(out_size), hi), frac)
    return M


def _tap_weights() -> np.ndarray:
    """wvec[w]: weight applied to input index w, whose (unique) consumer is
    output index w//2. Verifies the 2-tap stride-2 structure exactly."""
    M = _bilinear_matrix(H, S)  # [28, 56]
    wvec = np.zeros(H, np.float32)
    for w in range(H):
        wvec[w] = M[w // 2, w]
    M2 = np.zeros_like(M)
    for ow in range(S):
        M2[ow, 2 * ow] = wvec[2 * ow]
        M2[ow, 2 * ow + 1] = wvec[2 * ow + 1]
    assert np.abs(M - M2).max() <= 1e-6, "bilinear 2-tap structure violated"
    return wvec


_WVEC = _tap_weights()
# WF[p, h*56+w] = wvec[h]*wvec[w]  (full separable 2D weight map)
_WF_ROW = (np.repeat(_WVEC, W) * np.tile(_WVEC, H)).astype(np.float32)
WF_NP = np.ascontiguousarray(
    np.broadcast_to(_WF_ROW[None, :], (128, HW)), dtype=F16_NP
)


def _tap_tables(knn_inds: np.ndarray):
    """Flat hw indices of the 4 bilinear taps per knn point (for the host
    slice) + the matching tap-weight map (applied on device)."""
    knn = np.asarray(knn_inds)
    taps = np.zeros(4 * K, np.int64)
    wtap = np.zeros((128, 4 * K), np.float32)
    for k in range(knn.shape[0]):
        x = int(knn[k, 0])
        y = int(knn[k, 1])
        for j, (t, s) in enumerate(((0, 0), (0, 1), (1, 0), (1, 1))):
            taps[4 * k + j] = (2 * y + t) * W + (2 * x + s)
            wtap[:, 4 * k + j] = _WVEC[2 * y + t] * _WVEC[2 * x + s]
    return taps, np.ascontiguousarray(wtap.astype(F16_NP))


def _make_in_maps(f1: np.ndarray, f2: np.ndarray, knn_inds: np.ndarray):
    taps, wtap = _tap_tables(knn_inds)
    # f1 ships as fp8e4m3, prescaled by 16 to clear the denormal zone; the
    # 1/16 is folded into the psum->bf16 copy scale on device
    f1s = np.asarray(f1, np.float32).astype(F16_NP)
    # host-side shard/slice: tap columns of f2 (indexing only, no math);
    # sliced from the original f32 values and shipped as f16
    f2t = np.ascontiguousarray(
        np.asarray(f2, np.float32).reshape(B, C, HW)[:, :, taps].astype(F16_NP)
    )
    in_maps = []
    for c in range(NCORES):
        in_maps.append(
            {
                "f1": np.ascontiguousarray(f1s[c * BL : (c + 1) * BL]),
                "f2t": f2t[c * BL : (c + 1) * BL],
                "wf": WF_NP,
                "wtap": wtap,
            }
        )
    return in_maps


def _build(tc, out_ap, f1_ap, f2t_ap, wf_ap, wtap_ap):
    nc = tc.nc
    MS = __import__("concourse.bass", fromlist=["MemorySpace"]).MemorySpace

    from contextlib import ExitStack

    with ExitStack() as ctx:
        const = ctx.enter_context(tc.tile_pool(name="const", bufs=1))
        f2tp = ctx.enter_context(tc.tile_pool(name="f2tp", bufs=4))
        mpool = ctx.enter_context(tc.tile_pool(name="mpool", bufs=2))
        s1p = ctx.enter_context(tc.tile_pool(name="s1p", bufs=2))
        d2selp = ctx.enter_context(tc.tile_pool(name="d2selp", bufs=16))
        tf1p = ctx.enter_context(tc.tile_pool(name="tf1p", bufs=6))
        cbp = ctx.enter_context(tc.tile_pool(name="cbp", bufs=3))
        up = ctx.enter_context(tc.tile_pool(name="up", bufs=3))
        vp = ctx.enter_context(tc.tile_pool(name="vp", bufs=3))
        crawp = ctx.enter_context(tc.tile_pool(name="crawp", bufs=2))
        c28p = ctx.enter_context(tc.tile_pool(name="c28p", bufs=2))
        expbp = ctx.enter_context(tc.tile_pool(name="expbp", bufs=2))
        obp = ctx.enter_context(tc.tile_pool(name="obp", bufs=2))
        smallp = ctx.enter_context(tc.tile_pool(name="smallp", bufs=6))
        cpsp = ctx.enter_context(tc.tile_pool(name="cpsp", bufs=7, space=MS.PSUM))

        wf = const.tile([128, HW], F16, tag="wf")
        wtap = const.tile([128, 4 * K], F16, tag="wtap")
        nc.scalar.dma_start(wf[:], wf_ap)
        nc.scalar.dma_start(wtap[:], wtap_ap)

        # ---- f2 taps for ALL batches up front: tiny DMAs + combines, so
        # the PE matmul stream later only ever waits on f1 quarters ----
        d2sel_all = []
        for b in range(BL):
            f2t = f2tp.tile([128, NCB * 4 * K], F16, tag="f2t")
            nc.sync.dma_start(
                f2t.rearrange("c (g x) -> c g x", g=NCB),
                f2t_ap[b].rearrange("(g c) x -> c g x", c=128),
            )
            f2tv = f2t.rearrange("c (g x) -> c g x", g=NCB)
            d2sel_tiles = []
            for i in range(NCB):
                m = mpool.tile([128, 4 * K], F16, tag="m")
                nc.vector.tensor_mul(m[:], f2tv[:, i, :], wtap[:])
                mv = m.rearrange("c (k d) -> c k d", d=2)
                s1 = s1p.tile([128, 2 * K], F16, tag="s1")
                nc.vector.tensor_add(s1[:], mv[:, :, 0], mv[:, :, 1])
                s1v = s1.rearrange("c (k d) -> c k d", d=2)
                dsel = d2selp.tile([128, K], F16, tag="d2sel")
                nc.vector.tensor_add(dsel[:], s1v[:, :, 0], s1v[:, :, 1])
                d2sel_tiles.append(dsel)
            d2sel_all.append(d2sel_tiles)

        for b in range(BL):
            d2sel_tiles = d2sel_all[b]

            # ---- f1 load: four packed quarter-DMAs (16/16/16/8 h rows) ----
            tf1q = []
            for q, (r0, nr) in enumerate([(0, 16), (16, 16), (32, 24)]):
                t = tf1p.tile([128, NCB * nr * W], F16, tag=f"tf1q{q}")
                nc.sync.dma_start(
                    t.rearrange("c (g x) -> c g x", g=NCB),
                    f1_ap[b, :, r0 : r0 + nr, :].rearrange(
                        "(g c) h w -> c g (h w)", c=128
                    ),
                )
                tf1q.append(t.rearrange("c (g h w) -> c g h w", g=NCB, h=nr))

            # ---- correlation in 56x56 space + post-matmul downsample ----
            craw = crawp.tile([K, HW28], F32, tag="craw")
            cr3 = craw.rearrange("k (h w) -> k h w", h=S)
            for g in range(NG):  # 7 groups of 8 h-rows
                h0 = g * 8
                tv = tf1q[min(g // 2, 2)]
                hh = (g - 2 * min(g // 2, 2)) * 8
                cps = cpsp.tile([K, 8 * W], F32, tag="cps")
                for i in range(NCB):
                    nc.tensor.matmul(
                        cps[:],
                        d2sel_tiles[i][:],
                        tv[:, i, hh : hh + 8, :],
                        start=(i == 0),
                        stop=(i == NCB - 1),
                    )
                cb = cbp.tile([K, 8 * W], F16, tag="cb")
                if g % 2 == 0:
                    nc.scalar.copy(cb[:], cps[:])
                else:
                    nc.vector.tensor_copy(cb[:], cps[:])
                deng = nc.gpsimd if g in (2, 5) else nc.vector
                u = up.tile([K, 8 * W], F16, tag="u")
                deng.tensor_mul(u[:], cb[:], wf[0:K, h0 * W : (h0 + 8) * W])
                u3 = u.rearrange("k (h w) -> k h w", h=8)
                v = vp.tile([K, 4 * W], F16, tag="v")
                v3 = v.rearrange("k (h w) -> k h w", h=4)
                deng.tensor_add(v3, u3[:, 0:8:2, :], u3[:, 1:8:2, :])
                deng.tensor_add(
                    cr3[:, g * 4 : (g + 1) * 4, :],
                    v3[:, :, 0:W:2],
                    v3[:, :, 1:W:2],
                )

            # ---- exp, clamped accumulate (sum exp(relu(x)) = sum max(exp x, 1)),
            # reciprocal, then one fused relu(x)*10/denom ----
            expb = expbp.tile([K, HW28], F16, tag="expb")
            nc.scalar.activation(expb[:], craw[:], AF.Exp)
            expm = c28p.tile([K, HW28], F16, tag="expm")
            den = smallp.tile([K, 1], F32, tag="den")
            nc.vector.tensor_scalar(
                expm[:], expb[:], 1.0, 0.0, mybir.AluOpType.max,
                mybir.AluOpType.add, accum_out=den[:],
            )
            rec = smallp.tile([K, 1], F32, tag="rec")
            nc.vector.reciprocal(rec[:], den[:])
            rec10 = smallp.tile([K, 1], F32, tag="rec10")
            nc.vector.tensor_scalar_mul(rec10[:], rec[:], 10.0)
            ob = obp.tile([K, HW28], F32, tag="ob")
            nc.scalar.activation(ob[:], craw[:], AF.Relu, scale=rec10[:])
            # out DMA on the ACT HWDGE queue (inputs stream on SP)
            nc.scalar.dma_start(out_ap[b], ob[:])


_CACHE: dict = {}


def _get_nc():
    if "nc" in _CACHE:
        return _CACHE["nc"]
    nc = bacc.Bacc(
        "TRN2",
        target_bir_lowering=False,
        debug=False,
        enable_asserts=False,
        num_devices=NCORES,
    )
    f1 = nc.dram_tensor("f1", [BL, C, H, W], F16, kind="ExternalInput").ap()
    f2t = nc.dram_tensor("f2t", [BL, C, 4 * K], F16, kind="ExternalInput").ap()
    wf = nc.dram_tensor("wf", [128, HW], F16, kind="ExternalInput").ap()
    wtap = nc.dram_tensor("wtap", [128, 4 * K], F16, kind="ExternalInput").ap()
    out = nc.dram_tensor("out", [BL, K, HW28], F32, kind="ExternalOutput").ap()
    with tile.TileContext(nc) as tc:
        _build(tc, out, f1, f2t, wf, wtap)
    nc.compile()
    _CACHE["nc"] = nc
    return nc


def kernel(feature1, feature2, knn_inds):
    f1 = np.asarray(feature1, dtype=np.float32)
    f2 = np.asarray(feature2, dtype=np.float32)
    nc = _get_nc()
    in_maps = _make_in_maps(f1, f2, knn_inds)
    res = bass_utils.run_bass_kernel_spmd(nc, in_maps, core_ids=list(range(NCORES)))
    _CACHE["last_results"] = res
    out = np.concatenate([r["out"] for r in res.results], axis=0)
    return out.reshape(B, K, S, S)
